# revision 86
# baseline (speedup 1.0000x reference)
"""Linear attention (B=4, S=4096, D=1024, H=16) on 8 TRN2 NeuronCores.

Sharding: core = (batch, head-half): each core handles one batch's 8 heads.
 - x is host-transposed to xT [D, S] per batch so both operand orientations
   of every matmul come out of the tensor engine with no on-device transpose.
 - Wqkv column-sharded per head-half; Wo row-sharded; host sums the two
   partial (bf16) y's per batch (row-parallel unshard).

Per-core dataflow (S=4096 in 8 blocks of 512 tokens), all matmuls bf16
(fp32 PSUM accumulate; x/Wqkv/Wo host-cast to bf16):
  startup: x block 0 and Wq are DMA'd in interleaved per-128-contraction
      chunks into per-chunk tiles so the first matmul issues ~4us in; Wk/Wv
      follow in quarters, Wo/ones2 ride behind block 1's x.
  phase A: qkv projection:
      QT [512f, S] feature-major  (lhsT=Wq, rhs=xT)   -> elu+1 -> bf16 QT
      K,V [S, 512f] token-major   (lhsT=xT, rhs=Wkv)  -> elu+1(K), copy(V)
      (elu(x)+1 = min(exp(x),1) + relu(x): ACT Exp + DVE max + DVE stt;
      one manual LoadActFuncSet of set 6 covers Exp/Relu/Ln/Copy so the
      ACT engine never reloads tables)
  phase B: per head-pair [KV | K_sum^T] PSUM accumulation over all tokens
      (vst carries a ones column per pair so one matmul does both); B lags
      its block by one so the PE never waits on the kst elu chain. The last
      two blocks' QT runs after B, interleaved with the normalizer
      prologue: norm rows via zero-padded M=32 pair matmuls; Ln/Exp in
      waves (Exps no-sync-pinned after the wave's Lns) giving
      rcp = exp(-ln(norm+eps)) for all blocks while the PE chews QT(6/7).
  phase C/D main loop, all pair-packed, D skewed one block behind C:
      rcb(j): per-pair M=128 matmuls against a block-structured ones2
      broadcast both heads' rcp rows to [128, s] bf16 SBUF tiles, one
      block ahead of use (evictions alternate DVE/ACT)
      psc[128,s] = blockdiag(KV_h0, KV_h1)^T @ QT_pair -> both heads in
      one matmul; outT = psc * rcb as a single DVE tensor_tensor reading
      psc straight from PSUM (no separate eviction)
      D: y[s,:] (+)= outT^T @ Wo, evicted per 512-col chunk (ACT) with
      per-chunk DMA so the end-of-kernel drain is one chunk deep
"""

import numpy as np

import concourse.bacc as bacc
import concourse.mybir as mybir
import concourse.tile as tile
from concourse.bass_utils import run_bass_kernel_spmd
from concourse.tile import add_dep_helper

F32 = mybir.dt.float32
F32R = mybir.dt.float32r
BF16 = mybir.dt.bfloat16

P = 128
B, S, D = 4, 4096, 1024
H = 16
HD = 64
EPS = 1e-6

FSH = 512            # features per core for each of Q, K, V (8 heads)
KSUB = D // P        # 8 contraction subtiles
SBLK = 512           # tokens per block
NBLK = S // SBLK     # 8 blocks
TSUB = SBLK // P     # 4 token subtiles per block
NPAIR = 4            # head pairs per core
NHEAD = 8            # heads per core

_NC_CACHE = None


def build():
    nc = bacc.Bacc(target_bir_lowering=False)
    xT = nc.dram_tensor("xT", [D, S], BF16, kind="ExternalInput")
    wqkv = nc.dram_tensor("wqkv", [D, 3 * FSH], BF16, kind="ExternalInput")
    wo = nc.dram_tensor("wo", [FSH, D], BF16, kind="ExternalInput")
    ones2 = nc.dram_tensor("ones2", [P, P], F32R, kind="ExternalInput")
    y = nc.dram_tensor("y", [S, D], BF16, kind="ExternalOutput")

    xT_r = xT.rearrange("(ko p) s -> p ko s", p=P)        # [128, 8, 4096]
    wqkv_r = wqkv.rearrange("(ko p) f -> p ko f", p=P)    # [128, 8, 1536]
    wo_r = wo.rearrange("(fo p) n -> p fo n", p=P)        # [128, 4, 1024]
    y_r2 = y.rearrange(
        "(j th t p) n -> j p th t n", th=TSUB // 2, t=2, p=P
    )  # [8, 128, 2, 2, 1024]  (y bf16; host upcasts + sums the two halves)

    with tile.TileContext(nc) as tc:
        import contextlib

        with contextlib.ExitStack() as ctx:
            const = ctx.enter_context(tc.tile_pool(name="const", bufs=1))
            wpool = ctx.enter_context(tc.tile_pool(name="wpool", bufs=1))
            qtpool = ctx.enter_context(tc.tile_pool(name="qtpool", bufs=1))
            nrmpool = ctx.enter_context(tc.tile_pool(name="nrm", bufs=4))
            rcpool = ctx.enter_context(tc.tile_pool(name="rc", bufs=NBLK))
            rcbpool = ctx.enter_context(tc.tile_pool(name="rcb", bufs=1))

            # persistent SBUF (weight DMAs are issued inside phase A below,
            # chunked/ordered so the PE can start within ~2us of t=0).
            # Weights live in per-chunk tiles: readers of a tile written by
            # N DMAs wait for ALL N (tile-granular write coalescing), so
            # each DMA gets its own tile to keep the first matmuls unblocked.
            wq_sb = [
                wpool.tile([P, FSH], BF16, name=f"wq{k}") for k in range(KSUB)
            ]
            wkv_sb = [
                wpool.tile([P, 2, 2 * FSH], BF16, name=f"wkv{h}")
                for h in range(4)
            ]
            wo_sb = wpool.tile([P, FSH // P, D], BF16)
            qt_sb = qtpool.tile([P, FSH // P, S], BF16)   # feature-major Q
            # per-pair block-diagonal [[KV_h0, 0], [0, KV_h1]] (128x128):
            # one matmul against the stacked QT pair computes both heads
            lhsT2_sb = [
                qtpool.tile([P, P], BF16, name=f"lhsT2{p}") for p in range(NPAIR)
            ]
            # per-pair [Ksum_h0 | Ksum_h1 | zeros] (128 x 32): col 0 rows 0:64
            # = Ksum_even, col 1 rows 64:128 = Ksum_odd
            ksumpad_sb = [
                qtpool.tile([P, 32], BF16, name=f"ksp{p}") for p in range(NPAIR)
            ]
            # norm-path scratch (only partition row 64 is used; one buf each)

            # Preload ACT table set 6 (natural_log_exp_and_others): it holds
            # Exp, Relu, Ln AND Copy — every ACT function this kernel uses —
            # so the table-load insertion pass never has to switch sets
            # (otherwise it thrashes Exp<->Ln sets at ~1.3us per reload).
            nc.scalar.add_instruction(
                mybir.InstLoadActFuncSet(
                    name=nc.get_next_instruction_name(),
                    ins=[],
                    outs=[],
                    act_func_set_id=6,
                )
            )

            eps_sb = const.tile([P, 1], F32)
            nc.vector.memset(eps_sb, EPS)
            # ones2 (host-built): per 32-block, row 32k = [1x64 | 0x64],
            # row 32k+1 = [0x64 | 1x64] -- pair-broadcast stationary operand
            # (DMA'd behind block 1's x, long before phase C needs it)
            ones2_fr = const.tile([P, P], F32R)

            # ---------------- phase A + B ----------------
            with (
                tc.tile_pool(name="kvps", bufs=1, space="PSUM") as kvps_pool,
                tc.tile_pool(name="xin", bufs=3) as xpool,
                tc.tile_pool(name="xin0", bufs=1) as xpool0,
                tc.tile_pool(name="stage", bufs=3) as stpool,
                tc.tile_pool(name="paps", bufs=4, space="PSUM") as pa_ps,
                tc.tile_pool(name="etmp", bufs=4) as etpool,
            ):
                kvps = [
                    kvps_pool.tile([P, P + 1], F32, tag=f"kv{p}", name=f"kv{p}")
                    for p in range(NPAIR)
                ]
                kvstash = {}
                rcps = {}
                rcbs = {}

                def c_rcb(j, psum_pool, tag):
                    # rcp-broadcast for block j to bf16 SBUF; evictions
                    # alternate over the two PSUM-capable engines
                    rcb = rcbpool.tile(
                        [P, NPAIR, SBLK], BF16, tag=f"rcb{j}", name=f"rcb{j}"
                    )
                    rcbs[j] = rcb
                    rcpt = rcps.pop(j)
                    for p_ in range(NPAIR):
                        rb = 32 * p_
                        psr = psum_pool.tile([P, SBLK], F32, tag=tag, name="psr")
                        nc.tensor.matmul(
                            psr,
                            ones2_fr[rb : rb + 2, :],
                            rcpt[rb : rb + 2, :],
                            start=True,
                            stop=True,
                            tile_position=(rb, 0),
                        )
                        if p_ % 2 == 0:
                            nc.vector.tensor_copy(out=rcb[:, p_, :], in_=psr)
                        else:
                            nc.scalar.copy(out=rcb[:, p_, :], in_=psr)

                for j in range(NBLK):
                    if j == 0:
                        # DMA transfers drain in issue order; interleave x
                        # block 0 with Wq per contraction chunk so the first
                        # QT matmul group is fed as chunks land, then the
                        # K/V weights in two halves just ahead of their use.
                        xt0s = [
                            xpool0.tile(
                                [P, SBLK], BF16, tag=f"x0{k}", name=f"x0{k}"
                            )
                            for k in range(KSUB)
                        ]
                        for k in range(KSUB):
                            nc.sync.dma_start(
                                out=xt0s[k], in_=xT_r[:, k, 0:SBLK]
                            )
                            nc.sync.dma_start(
                                out=wq_sb[k], in_=wqkv_r[:, k, 0:FSH]
                            )
                        for kq in range(4):
                            ks = slice(kq * 2, kq * 2 + 2)
                            nc.sync.dma_start(
                                out=wkv_sb[kq],
                                in_=wqkv_r[:, ks, FSH : 3 * FSH],
                            )
                        xta = lambda k: xt0s[k]
                    else:
                        xt = xpool.tile([P, KSUB, SBLK], BF16, tag="xt")
                        nc.sync.dma_start(
                            out=xt, in_=xT_r[:, :, j * SBLK : (j + 1) * SBLK]
                        )
                        xta = lambda k, xt=xt: xt[:, k]
                    if j == 1:
                        # phase C/D constants ride behind block 1's x
                        nc.sync.dma_start(out=wo_sb, in_=wo_r)
                        nc.sync.dma_start(out=ones2_fr, in_=ones2[:])

                    def qt_phase(j, xta):
                        # QT: 4 feature blocks of 128
                        for f in range(FSH // P):
                            ps = pa_ps.tile([P, SBLK], F32, tag="pa")
                            for k in range(KSUB):
                                nc.tensor.matmul(
                                    ps,
                                    wq_sb[k][:, f * P : (f + 1) * P],
                                    xta(k),
                                    start=(k == 0),
                                    stop=(k == KSUB - 1),
                                )
                            e = etpool.tile([P, SBLK], F32, tag="e")
                            nc.scalar.activation(
                                out=e,
                                in_=ps,
                                func=mybir.ActivationFunctionType.Exp,
                            )
                            r = etpool.tile([P, SBLK], F32, tag="r")
                            nc.vector.tensor_scalar_max(r, ps, 0.0)
                            nc.vector.scalar_tensor_tensor(
                                out=qt_sb[:, f, j * SBLK : (j + 1) * SBLK],
                                in0=e,
                                scalar=1.0,
                                in1=r,
                                op0=mybir.AluOpType.min,
                                op1=mybir.AluOpType.add,
                            )

                    def kv_phase(j, xta):
                        # K, V token-major per 128-token subtile.
                        # vst carries a ones column per head-pair slot so one
                        # matmul accumulates both KV and K_sum^T.
                        kst = stpool.tile([P, TSUB, FSH], BF16, tag="kst")
                        vst = stpool.tile(
                            [P, TSUB, NPAIR, P + 1], BF16, tag="vst"
                        )
                        nc.vector.memset(vst[:, :, :, P : P + 1], 1.0)
                        for t in range(TSUB):
                            psk = pa_ps.tile([P, FSH], F32, tag="pa")
                            psv = pa_ps.tile([P, FSH], F32, tag="pa")
                            for k in range(KSUB):
                                nc.tensor.matmul(
                                    psk,
                                    xta(k)[:, t * P : (t + 1) * P],
                                    wkv_sb[k // 2][:, k % 2, 0:FSH],
                                    start=(k == 0),
                                    stop=(k == KSUB - 1),
                                )
                                nc.tensor.matmul(
                                    psv,
                                    xta(k)[:, t * P : (t + 1) * P],
                                    wkv_sb[k // 2][:, k % 2, FSH : 2 * FSH],
                                    start=(k == 0),
                                    stop=(k == KSUB - 1),
                                )
                            e = etpool.tile([P, SBLK], F32, tag="e")
                            nc.scalar.activation(
                                out=e,
                                in_=psk,
                                func=mybir.ActivationFunctionType.Exp,
                            )
                            r = etpool.tile([P, SBLK], F32, tag="r")
                            nc.vector.tensor_scalar_max(r, psk, 0.0)
                            nc.vector.scalar_tensor_tensor(
                                out=kst[:, t, :],
                                in0=e,
                                scalar=1.0,
                                in1=r,
                                op0=mybir.AluOpType.min,
                                op1=mybir.AluOpType.add,
                            )

                            nc.scalar.copy(out=vst[:, t, :, 0:P], in_=psv)
                        return kst, vst

                    def b_phase(j, kst, vst):
                        # accumulate [KV | K_sum^T] into persistent psums
                        first = j == 0
                        last = j == NBLK - 1
                        for t in range(TSUB):
                            for p_ in range(NPAIR):
                                nc.tensor.matmul(
                                    kvps[p_],
                                    kst[:, t, p_ * P : (p_ + 1) * P],
                                    vst[:, t, p_, :],
                                    start=(first and t == 0),
                                    stop=(last and t == TSUB - 1),
                                )

                    # b_phase(j) is emitted one block late so its small
                    # matmuls never make the PE wait on the kst elu chain.
                    # The LAST TWO blocks' QT runs after all of B, so the PE
                    # has ~13us of projection matmuls to chew while the ACT
                    # engine works through the normalizer prologue's Ln/Exp
                    # chain (which only needs qt blocks 0..5 at first).
                    if j < NBLK - 2:
                        qt_phase(j, xta)
                        if j >= 1:
                            b_phase(j - 1, *kvstash.pop(j - 1))
                        kvstash[j] = kv_phase(j, xta)
                    elif j == NBLK - 2:
                        kvstash[j] = kv_phase(j, xta)
                        b_phase(j - 1, *kvstash.pop(j - 1))
                        qt_late = [(j, xta)]
                    else:
                        kvstash[j] = kv_phase(j, xta)
                        b_phase(j - 1, *kvstash.pop(j - 1))
                        b_phase(j, *kvstash.pop(j))
                        # KV-state evictions on DVE (PSUM reads; gpsimd
                        # can't): overlap the late QT matmuls
                        for p_ in range(NPAIR):
                            nc.gpsimd.memset(ksumpad_sb[p_], 0.0)
                            nc.vector.tensor_copy(
                                out=ksumpad_sb[p_][0:HD, 0:1],
                                in_=kvps[p_][0:HD, P : P + 1],
                            )
                            nc.vector.tensor_copy(
                                out=ksumpad_sb[p_][HD:P, 1:2],
                                in_=kvps[p_][HD:P, P : P + 1],
                            )
                            nc.gpsimd.memset(lhsT2_sb[p_], 0.0)
                            nc.vector.tensor_copy(
                                out=lhsT2_sb[p_][0:HD, 0:HD],
                                in_=kvps[p_][0:HD, 0:HD],
                            )
                            nc.vector.tensor_copy(
                                out=lhsT2_sb[p_][HD:P, HD:P],
                                in_=kvps[p_][HD:P, HD:P],
                            )
                        qt_late.append((j, xta))

                        # normalizer prologue interleaved with the two late
                        # QT blocks: psn borrows pa_ps banks; while ACT
                        # grinds a wave's Ln/Exp chain (~6us), the PE chews
                        # a deferred QT block instead of idling. Waves of 4
                        # with Exps pinned after the wave's last Ln keep ACT
                        # table loads at 4 instead of 16.
                        def norm_wave(jns):
                            nrmts = {}
                            last_ln = None
                            for jn in jns:
                                psn = pa_ps.tile(
                                    [P, SBLK], F32, tag="pa", name="psn"
                                )
                                for p_ in range(NPAIR):
                                    nc.tensor.matmul(
                                        psn[32 * p_ : 32 * p_ + 32, :],
                                        ksumpad_sb[p_],
                                        qt_sb[:, p_, jn * SBLK : (jn + 1) * SBLK],
                                        start=True,
                                        stop=True,
                                        tile_position=(0, 32 * p_),
                                    )
                                nrmt = nrmpool.tile(
                                    [P, SBLK], F32, tag="nt", name="nrmt"
                                )
                                last_ln = nc.scalar.activation(
                                    out=nrmt,
                                    in_=psn,
                                    func=mybir.ActivationFunctionType.Ln,
                                    bias=eps_sb,
                                )
                                nrmts[jn] = nrmt
                            for jn in jns:
                                rcpt = rcpool.tile(
                                    [P, SBLK], F32R, tag="rc", name="rcpt"
                                )
                                with nc.allow_low_precision(
                                    reason="fp32r is 32-bit; matmul operand"
                                ):
                                    ei = nc.scalar.activation(
                                        out=rcpt,
                                        in_=nrmts.pop(jn),
                                        func=mybir.ActivationFunctionType.Exp,
                                        scale=-1.0,
                                    )
                                add_dep_helper(
                                    ei.ins,
                                    last_ln.ins,
                                    sync=False,
                                    reason="batch Ln before Exp per wave",
                                )
                                rcps[jn] = rcpt

                        # each wave only uses qt blocks already computed;
                        # the late QT blocks are the PE filler under each
                        # wave's ACT chain
                        norm_wave(range(0, 4))
                        qt_phase(*qt_late[0])
                        norm_wave(range(4, 7))
                        qt_phase(*qt_late[1])
                        norm_wave([7])

            # ---------------- phase C + D ----------------
            # Prologue computes EVERYTHING normalizer-related for all 8
            # blocks (qt_sb/ksumpad are complete once phase A ends):
            # psn matmuls -> Ln x8 -> Exp x8 -> rcp-broadcast matmuls -> bf16
            # rcpb tiles in SBUF. The main loop's apply is then a single DVE
            # tensor_tensor reading psc STRAIGHT from PSUM (one PSUM operand
            # is allowed), so psc needs no eviction at all and the only
            # per-block evictions left are the 4 ysb copies on ACT.
            # PSUM banks: pc 4 + pr 2 + py 2 = 8 (prologue: psn on py, psr
            # on pr; pc idle until the main loop).
            with (
                tc.tile_pool(name="pcps", bufs=2, space="PSUM") as pc_ps,
                tc.tile_pool(name="prps", bufs=4, space="PSUM") as pr_ps,
                tc.tile_pool(name="pyps", bufs=2, space="PSUM") as py_ps,
                tc.tile_pool(name="cd", bufs=2) as cdpool,
                tc.tile_pool(name="yout", bufs=2) as ypool,
            ):
                pscs = {}
                outts = {}

                def c_psc(j, ps_):
                    if j not in pscs:
                        pscs[j] = {}
                    for p_ in ps_:
                        psc = pc_ps.tile([P, SBLK], F32, tag="pc", name="psc")
                        nc.tensor.matmul(
                            psc,
                            lhsT2_sb[p_],
                            qt_sb[:, p_, j * SBLK : (j + 1) * SBLK],
                            start=True,
                            stop=True,
                        )
                        pscs[j][p_] = psc

                def c_apply(j, ps_):
                    if j not in outts:
                        outts[j] = cdpool.tile(
                            [P, FSH // P, SBLK], BF16, tag="outt", name="outt"
                        )
                    outt = outts[j]
                    rcb = rcbs[j]
                    for p_ in ps_:
                        nc.vector.tensor_tensor(
                            out=outt[:, p_, :],
                            in0=pscs[j].pop(p_),
                            in1=rcb[:, p_, :],
                            op=mybir.AluOpType.mult,
                        )

                def d_half(j, th):
                    outt = outts[j]
                    ysb = ypool.tile([P, 2, D], BF16, tag="ysb", name="ysb")
                    final = j == NBLK - 1 and th == TSUB // 2 - 1
                    for t2 in range(2):
                        t = th * 2 + t2
                        for nb in range(D // 512):
                            psy = py_ps.tile([P, 512], F32, tag="py", name="psy")
                            for fs in range(FSH // P):
                                nc.tensor.matmul(
                                    psy,
                                    outt[:, fs, t * P : (t + 1) * P],
                                    wo_sb[:, fs, nb * 512 : (nb + 1) * 512],
                                    start=(fs == 0),
                                    stop=(fs == FSH // P - 1),
                                )
                            sl = slice(nb * 512, (nb + 1) * 512)
                            # fine-grained stores: each chunk's DMA overlaps
                            # the next chunk's matmuls, shortening the
                            # end-of-kernel drain. The final half-block's
                            # copies alternate DVE/ACT so the last few
                            # evictions drain in parallel, not serially.
                            if final and (t2 + nb) % 2 == 0:
                                nc.vector.tensor_copy(out=ysb[:, t2, sl], in_=psy)
                            else:
                                nc.scalar.copy(out=ysb[:, t2, sl], in_=psy)
                            nc.sync.dma_start(
                                out=y_r2[j, :, th, t2][:, sl],
                                in_=ysb[:, t2, sl],
                            )
                    if th == TSUB // 2 - 1:
                        outts.pop(j)

                # interleaved emission: the previous block's Wo matmuls slot
                # between this block's psc matmuls and applies so every
                # engine's work arrives in dribbles rather than clumps; each
                # rcb broadcast runs one block ahead of its apply.
                c_rcb(0, pr_ps, "pr")
                for j in range(NBLK):
                    c_psc(j, [0, 1])
                    if j >= 1:
                        d_half(j - 1, 0)
                    c_apply(j, [0, 1])
                    c_psc(j, [2, 3])
                    if j + 1 < NBLK:
                        c_rcb(j + 1, pr_ps, "pr")
                    if j >= 1:
                        d_half(j - 1, 1)
                    c_apply(j, [2, 3])
                d_half(NBLK - 1, 0)
                d_half(NBLK - 1, 1)

    nc.compile()
    return nc


def _prep_inputs(x, Wqkv, Wo):
    import ml_dtypes

    x = np.ascontiguousarray(x, dtype=np.float32)
    Wqkv = np.ascontiguousarray(Wqkv, dtype=np.float32)
    Wo = np.ascontiguousarray(Wo, dtype=np.float32)
    in_maps = []
    for b in range(B):
        xT = np.ascontiguousarray(x[b].T).astype(ml_dtypes.bfloat16)  # [D, S]
        for hh in range(2):
            cols = slice(hh * FSH, (hh + 1) * FSH)
            wq = Wqkv[:, 0 * D :][:, cols]
            wk = Wqkv[:, 1 * D :][:, cols]
            wv = Wqkv[:, 2 * D :][:, cols]
            wqkv_sh = np.ascontiguousarray(
                np.concatenate([wq, wk, wv], axis=1)
            ).astype(ml_dtypes.bfloat16)
            wo_sh = np.ascontiguousarray(Wo[hh * FSH : (hh + 1) * FSH, :]).astype(
                ml_dtypes.bfloat16
            )
            ones2 = np.zeros((128, 128), dtype=np.float32)
            for k in range(4):
                ones2[32 * k, 0:64] = 1.0
                ones2[32 * k + 1, 64:128] = 1.0
            in_maps.append(
                {"xT": xT, "wqkv": wqkv_sh, "wo": wo_sh, "ones2": ones2}
            )
    return in_maps


def kernel(x, Wqkv, Wo):
    global _NC_CACHE
    if _NC_CACHE is None:
        _NC_CACHE = build()
    nc = _NC_CACHE
    in_maps = _prep_inputs(x, Wqkv, Wo)
    res = run_bass_kernel_spmd(nc, in_maps, list(range(2 * B))).results
    y = np.empty((B, S, D), dtype=np.float32)
    for b in range(B):
        y[b] = res[2 * b]["y"].astype(np.float32) + res[2 * b + 1]["y"].astype(
            np.float32
        )
    return y

